# revision 2
# baseline (speedup 1.0000x reference)
"""Trainium2 Bass kernel for nn_AttentionBlock (B=4, H=W=64, C=256, D=32).

Sharding: 8 shards = 4 samples x 2 query-halves. Each core gets the full
sample (rows reordered so its 2048 query rows come first), computes K and
the fused V@Wo projection for all 4096 keys, and attention + residual for
its 2048 queries. No collectives needed.

Key structure (v2):
  - x is transposed on the host (layout prep), so no on-device transpose
    phase: xT arrives channel-major [128, 2, 4096].
  - wo is folded into the value projection on the host: W = x @ (wv@wo)
    (+ bv@wo). Attention output then IS the final projection:
    out = (exp(S) @ W) / denom + x_residual. No epilogue transpose or
    output-projection matmuls.
  - Scores (contraction D=32) use 2-way PE row tiling: two concurrent
    matmuls at tile_position (0,0)/(32,0) with q/k replicated across
    partition bands (via column-replicated wq/wk). Score PSUM is a
    6-bank ring (3 pairs x 2 banks); exp reads a retired pair while the
    PE writes the other pairs, so no PSUM bank collisions/stalls.
  - Softmax denominator via a ones-column appended to W (free dim 257).

Self-contained: hardcodes shapes, imports only /opt/trn_rl_repo concourse.
"""

import sys

if "/opt/trn_rl_repo" not in sys.path:
    sys.path.insert(0, "/opt/trn_rl_repo")

import numpy as np
import ml_dtypes

BF16 = ml_dtypes.bfloat16

# Problem constants
B, HH, WW, C = 4, 64, 64, 256
D = 32
N = HH * WW          # 4096 keys per sample
NQ = N // 2          # 2048 queries per core
NCORES = 8
KC = N // 128        # 32 key chunks

_compiled_cache = {}


def _build(use_bias: bool):
    from contextlib import ExitStack
    from concourse import bacc, tile, mybir

    f32 = mybir.dt.float32
    bf = mybir.dt.bfloat16

    nc = bacc.Bacc("TRN2", target_bir_lowering=False, debug=False, num_devices=NCORES)

    xT_d = nc.dram_tensor("xT", [128, 2, N], bf, kind="ExternalInput")
    xq32_d = nc.dram_tensor("xq32", [NQ, C], f32, kind="ExternalInput")
    wqa_d = nc.dram_tensor("wqa_rep", [257, 128], bf, kind="ExternalInput")
    wka_d = nc.dram_tensor("wka_rep", [257, 128], bf, kind="ExternalInput")
    wvoa_d = nc.dram_tensor("wvoa", [257, 256], bf, kind="ExternalInput")
    out_d = nc.dram_tensor("out", [NQ, C], f32, kind="ExternalOutput")

    Exp = mybir.ActivationFunctionType.Exp
    Add = mybir.AluOpType.add
    Mult = mybir.AluOpType.mult

    with tile.TileContext(nc) as tc:
        with ExitStack() as ctx:
            const = ctx.enter_context(tc.tile_pool(name="const", bufs=1))
            big = ctx.enter_context(tc.tile_pool(name="big", bufs=1))
            xbp = ctx.enter_context(tc.tile_pool(name="xbp", bufs=3))
            expp = ctx.enter_context(tc.tile_pool(name="expp", bufs=4))
            small = ctx.enter_context(tc.tile_pool(name="small", bufs=2))
            # PSUM: scores ring 6 banks + shared 2 banks = 8 banks exactly.
            ps_sc = ctx.enter_context(tc.tile_pool(name="ps_sc", bufs=1, space="PSUM"))
            ps2 = ctx.enter_context(tc.tile_pool(name="ps2", bufs=2, space="PSUM"))

            # ---- weights ----
            wq0 = const.tile([128, 128], bf, tag="wq0")
            wq1 = const.tile([128, 128], bf, tag="wq1")
            wk0 = const.tile([128, 128], bf, tag="wk0")
            wk1 = const.tile([128, 128], bf, tag="wk1")
            wvo0 = const.tile([128, 256], bf, tag="wvo0")
            wvo1 = const.tile([128, 256], bf, tag="wvo1")
            nc.sync.dma_start(out=wq0[:], in_=wqa_d[0:128, :])
            nc.sync.dma_start(out=wq1[:], in_=wqa_d[128:256, :])
            nc.sync.dma_start(out=wk0[:], in_=wka_d[0:128, :])
            nc.sync.dma_start(out=wk1[:], in_=wka_d[128:256, :])
            nc.sync.dma_start(out=wvo0[:], in_=wvoa_d[0:128, :])
            nc.sync.dma_start(out=wvo1[:], in_=wvoa_d[128:256, :])
            if use_bias:
                ones_row = const.tile([1, 512], bf, tag="ones_row")
                nc.gpsimd.memset(ones_row[:], 1.0)
                wqb = const.tile([1, 128], bf, tag="wqb")
                wkb = const.tile([1, 128], bf, tag="wkb")
                wvob = const.tile([1, 256], bf, tag="wvob")
                nc.sync.dma_start(out=wqb[:], in_=wqa_d[256:257, :])
                nc.sync.dma_start(out=wkb[:], in_=wka_d[256:257, :])
                nc.sync.dma_start(out=wvob[:], in_=wvoa_d[256:257, :])

            # Persistent SBUF: replicated qT/kT (4 bands of 32 partitions each
            # hold identical data, enabling PE row tiling), and W (= V@Wo) rows
            # with a ones column at 256 for the softmax denominator.
            qT4 = big.tile([128, NQ], bf, tag="qT4")
            kT4 = big.tile([128, N], bf, tag="kT4")
            wsb = big.tile([128, 16, 2, 260], bf, tag="wsb")
            nc.vector.memset(wsb[:, :, :, 256:257], 1.0)

            # ---- phase B: per 512-column chunk of xT: DMA, q/k proj, W proj ----
            for s in range(8):
                xb = xbp.tile([128, 2, 512], bf, tag="xb")
                nc.sync.dma_start(out=xb[:], in_=xT_d[:, :, 512 * s : 512 * s + 512])
                if s < 4:
                    pq = ps2.tile([128, 512], f32, tag="x", name=f"pq{s}")
                    nc.tensor.matmul(pq[:], wq0[:], xb[:, 0, :], start=True, stop=False)
                    nc.tensor.matmul(pq[:], wq1[:], xb[:, 1, :], start=False, stop=not use_bias)
                    if use_bias:
                        nc.tensor.matmul(pq[:], wqb[:], ones_row[:], start=False, stop=True)
                    nc.vector.tensor_copy(qT4[:, 512 * s : 512 * s + 512], pq[:])
                pk = ps2.tile([128, 512], f32, tag="x", name=f"pk{s}")
                nc.tensor.matmul(pk[:], wk0[:], xb[:, 0, :], start=True, stop=False)
                nc.tensor.matmul(pk[:], wk1[:], xb[:, 1, :], start=False, stop=not use_bias)
                if use_bias:
                    nc.tensor.matmul(pk[:], wkb[:], ones_row[:], start=False, stop=True)
                nc.vector.tensor_copy(kT4[:, 512 * s : 512 * s + 512], pk[:])
                # W = x @ (wv@wo): 4 key chunks of 128 per xb, 2 chunks per psum
                for half in range(2):
                    pw = ps2.tile([128, 512], f32, tag="x", name=f"pw{s}_{half}")
                    for j in range(2):
                        off = 128 * (2 * half + j)
                        nc.tensor.matmul(pw[:, 256 * j : 256 * j + 256], xb[:, 0, off : off + 128], wvo0[:], start=True, stop=False)
                        nc.tensor.matmul(pw[:, 256 * j : 256 * j + 256], xb[:, 1, off : off + 128], wvo1[:], start=False, stop=not use_bias)
                        if use_bias:
                            nc.tensor.matmul(pw[:, 256 * j : 256 * j + 256], ones_row[:, 0:128], wvob[:], start=False, stop=True)
                    P = 2 * s + half
                    # evacuate on scalar engine (vector handles q/k + epilogue)
                    nc.scalar.copy(wsb[:, P, :, 0:256], pw[:])

            # ---- phase C/D: scores (2-way row-tiled) -> exp -> attend ----
            # Superstep u of group g: 4 key chunks m = 4u + 2i + cc
            # (i = substep, cc = concurrent row-tile). Score psum is a 6-bank
            # ring [128, 6, 512]: bank 2r+cc (r = superstep % 3), col-half i.
            pst = ps_sc.tile([128, 6, 512], f32, tag="sc")

            def epilogue(qb, pa_t):
                rec = small.tile([128, 1], f32, tag="rec")
                nc.vector.reciprocal(rec[:], pa_t[:, 256:257])
                xq = small.tile([128, 256], f32, tag="xq", bufs=3)
                nc.sync.dma_start(out=xq[:], in_=xq32_d[128 * qb : 128 * qb + 128, :])
                sc = small.tile([128, 256], f32, tag="sc2")
                nc.vector.tensor_scalar(sc[:], pa_t[:, 0:256], rec[:], None, Mult)
                ot = small.tile([128, 256], f32, tag="ot", bufs=3)
                nc.vector.tensor_tensor(ot[:], sc[:], xq[:], Add)
                nc.sync.dma_start(out=out_d[128 * qb : 128 * qb + 128, :], in_=ot[:])

            pa_tiles = {}
            prev = None  # (et, g, u)
            NSS = 8 * 8  # 8 groups x 8 supersteps
            for idx in range(NSS + 1):
                if idx < NSS:
                    g, u = divmod(idx, 8)
                    if u == 0:
                        pa_tiles[2 * g] = ps2.tile([128, 512], f32, tag="x", name=f"pa{2 * g}")
                        pa_tiles[2 * g + 1] = ps2.tile([128, 512], f32, tag="x", name=f"pa{2 * g + 1}")
                    r = idx % 3
                    for i in range(2):
                        for cc in range(2):
                            m = 4 * u + 2 * i + cc
                            nc.tensor.matmul(
                                pst[:, 2 * r + cc, 256 * i : 256 * i + 256],
                                kT4[32 * cc : 32 * cc + 32, 128 * m : 128 * m + 128],
                                qT4[32 * cc : 32 * cc + 32, 256 * g : 256 * g + 256],
                                start=True,
                                stop=True,
                                tile_position=(32 * cc, 0),
                            )
                # attend with previous superstep's exp tile (keeps PE busy during exp)
                if prev is not None:
                    et_p, g_p, u_p = prev
                    for i in range(2):
                        for cc in range(2):
                            m = 4 * u_p + 2 * i + cc
                            for h in range(2):
                                nc.tensor.matmul(
                                    pa_tiles[2 * g_p + h][:, 0:257],
                                    et_p[:, 512 * cc + 256 * i + 128 * h : 512 * cc + 256 * i + 128 * h + 128],
                                    wsb[:, m // 2, cc, 0:257],
                                    start=(m == 0),
                                    stop=(m == KC - 1),
                                )
                    if u_p == 7:
                        for h in range(2):
                            epilogue(2 * g_p + h, pa_tiles[2 * g_p + h])
                            del pa_tiles[2 * g_p + h]
                if idx < NSS:
                    et = expp.tile([128, 1024], bf, tag="e")
                    nc.scalar.activation(et[:], pst[:, 2 * r : 2 * r + 2, 0:512], Exp)
                    prev = (et, g, u)

    nc.compile()
    return nc


def _get_compiled(use_bias: bool):
    key = bool(use_bias)
    if key not in _compiled_cache:
        _compiled_cache[key] = _build(use_bias)
    return _compiled_cache[key]


def _prep(x, wq, bq, wk, bk, wv, bv, wo, bo):
    xf = np.ascontiguousarray(np.asarray(x, dtype=np.float32)).reshape(B, N, C)
    wq = np.asarray(wq, np.float32)
    bq = np.asarray(bq, np.float32)
    wk = np.asarray(wk, np.float32)
    bk = np.asarray(bk, np.float32)
    wv = np.asarray(wv, np.float32)
    bv = np.asarray(bv, np.float32)
    wo = np.asarray(wo, np.float32)
    bo = np.asarray(bo, np.float32)

    use_bias = not (
        np.all(bq == 0) and np.all(bk == 0) and np.all(bv == 0) and np.all(bo == 0)
    )

    scale = np.float32(1.0 / np.sqrt(np.float32(D)))
    wqa = np.concatenate([wq, bq[None, :]], 0) * scale  # fold softmax scale into q
    wka = np.concatenate([wk, bk[None, :]], 0)
    wqa_rep = np.ascontiguousarray(np.tile(wqa, (1, 4))).astype(BF16)  # [257, 128]
    wka_rep = np.ascontiguousarray(np.tile(wka, (1, 4))).astype(BF16)
    # fold wo into the value projection: W = x @ (wv@wo) + bv@wo
    wvo = wv @ wo
    bvo = bv @ wo
    wvoa = np.ascontiguousarray(
        np.concatenate([wvo, bvo[None, :]], 0)
    ).astype(BF16)  # [257, 256]

    in_maps = []
    for core in range(NCORES):
        b, h = divmod(core, 2)
        if h == 0:
            xo = xf[b]
        else:
            xo = np.concatenate([xf[b, NQ:], xf[b, :NQ]], 0)
        # channel-major transpose on host: [256, 4096] -> [128, 2, 4096]
        xT = np.ascontiguousarray(
            xo.T.reshape(2, 128, N).transpose(1, 0, 2).astype(BF16)
        )
        xq = np.ascontiguousarray(xo[:NQ])
        if use_bias:
            xq = xq + bo[None, :]
        in_maps.append(
            {
                "xT": xT,
                "xq32": xq,
                "wqa_rep": wqa_rep,
                "wka_rep": wka_rep,
                "wvoa": wvoa,
            }
        )
    return in_maps, use_bias


def _gather(results):
    out = np.empty((B, N, C), np.float32)
    for core in range(NCORES):
        b, h = divmod(core, 2)
        out[b, NQ * h : NQ * (h + 1)] = results[core]["out"]
    return out.reshape(B, HH, WW, C)


def kernel(x, wq, bq, wk, bk, wv, bv, wo, bo):
    from concourse.bass_utils import run_bass_kernel_spmd

    in_maps, use_bias = _prep(x, wq, bq, wk, bk, wv, bv, wo, bo)
    nc = _get_compiled(use_bias)
    res = run_bass_kernel_spmd(nc, in_maps, core_ids=list(range(NCORES)))
    return _gather(res.results)


def _ensure_ntff_hook():
    """The agent image's antenv stub lacks axon_hooks; synthesize it so
    run_bass_kernel_spmd(trace=True) can NTFF-profile via libaxon_pjrt."""
    import types

    try:
        from antenv.axon_hooks import get_axon_ntff_profile_hook  # noqa: F401
        return
    except ImportError:
        pass
    import antenv
    from trn_agent_boot.trn_boot import _ntff_profile_via_ctypes

    mod = types.ModuleType("antenv.axon_hooks")
    state = {"h": _ntff_profile_via_ctypes("/opt/axon/libaxon_pjrt.so")}
    mod.get_axon_ntff_profile_hook = lambda: state["h"]
    mod.set_axon_ntff_profile_hook = lambda h: state.__setitem__("h", h)
    sys.modules["antenv.axon_hooks"] = mod
    antenv.axon_hooks = mod


def run_traced(inputs, **kw):
    """For test.py: run with NTFF profiling; returns (output, BassKernelResults)."""
    from concourse.bass_utils import run_bass_kernel_spmd

    _ensure_ntff_hook()

    in_maps, use_bias = _prep(**inputs)
    nc = _get_compiled(use_bias)
    res = run_bass_kernel_spmd(nc, in_maps, core_ids=list(range(NCORES)), trace=True, **kw)
    return _gather(res.results), res


# revision 5
# speedup vs baseline: 1.2037x; 1.2037x over previous
"""Trainium2 Bass kernel for nn_AttentionBlock (B=4, H=W=64, C=256, D=32).

Sharding: 8 shards = 4 samples x 2 query-halves. Each core gets the full
sample (rows reordered so its 2048 query rows come first), computes K and
the fused V@Wo projection for all 4096 keys, and attention + residual for
its 2048 queries. No collectives needed.

Key structure (v2):
  - x is transposed on the host (layout prep), so no on-device transpose
    phase: xT arrives channel-major [128, 2, 4096].
  - wo is folded into the value projection on the host: W = x @ (wv@wo)
    (+ bv@wo). Attention output then IS the final projection:
    out = (exp(S) @ W) / denom + x_residual. No epilogue transpose or
    output-projection matmuls.
  - Scores (contraction D=32) use 2-way PE row tiling: two concurrent
    matmuls at tile_position (0,0)/(32,0) with q/k replicated across
    partition bands (via column-replicated wq/wk). Score PSUM is a
    6-bank ring (3 pairs x 2 banks); exp reads a retired pair while the
    PE writes the other pairs, so no PSUM bank collisions/stalls.
  - Softmax denominator via a ones-column appended to W (free dim 257).

Self-contained: hardcodes shapes, imports only /opt/trn_rl_repo concourse.
"""

import sys

if "/opt/trn_rl_repo" not in sys.path:
    sys.path.insert(0, "/opt/trn_rl_repo")

import numpy as np
import ml_dtypes

BF16 = ml_dtypes.bfloat16

# Problem constants
B, HH, WW, C = 4, 64, 64, 256
D = 32
N = HH * WW          # 4096 keys per sample
NQ = N // 2          # 2048 queries per core
NCORES = 8
KC = N // 128        # 32 key chunks

_compiled_cache = {}


def _build(use_bias: bool):
    from contextlib import ExitStack
    from concourse import bacc, tile, mybir

    f32 = mybir.dt.float32
    bf = mybir.dt.bfloat16

    nc = bacc.Bacc("TRN2", target_bir_lowering=False, debug=False, num_devices=NCORES)

    xT_d = nc.dram_tensor("xT", [128, 2, N], bf, kind="ExternalInput")
    xq32_d = nc.dram_tensor("xq32", [NQ, C], f32, kind="ExternalInput")
    wqa_d = nc.dram_tensor("wqa_rep", [257, 128], bf, kind="ExternalInput")
    wka_d = nc.dram_tensor("wka_rep", [257, 128], bf, kind="ExternalInput")
    wvoa_d = nc.dram_tensor("wvoa", [257, 256], bf, kind="ExternalInput")
    out_d = nc.dram_tensor("out", [NQ, C], f32, kind="ExternalOutput")

    Exp = mybir.ActivationFunctionType.Exp
    Add = mybir.AluOpType.add
    Mult = mybir.AluOpType.mult

    with tile.TileContext(nc) as tc:
        with ExitStack() as ctx:
            const = ctx.enter_context(tc.tile_pool(name="const", bufs=1))
            big = ctx.enter_context(tc.tile_pool(name="big", bufs=1))
            xbp = ctx.enter_context(tc.tile_pool(name="xbp", bufs=3))
            expp = ctx.enter_context(tc.tile_pool(name="expp", bufs=4))
            small = ctx.enter_context(tc.tile_pool(name="small", bufs=2))
            # PSUM: scores ring 6 banks + shared 2 banks = 8 banks exactly.
            ps_sc = ctx.enter_context(tc.tile_pool(name="ps_sc", bufs=1, space="PSUM"))
            ps2 = ctx.enter_context(tc.tile_pool(name="ps2", bufs=2, space="PSUM"))

            # ---- weights ----
            wq0 = const.tile([128, 128], bf, tag="wq0")
            wq1 = const.tile([128, 128], bf, tag="wq1")
            wk0 = const.tile([128, 128], bf, tag="wk0")
            wk1 = const.tile([128, 128], bf, tag="wk1")
            wvo0 = const.tile([128, 256], bf, tag="wvo0")
            wvo1 = const.tile([128, 256], bf, tag="wvo1")
            nc.sync.dma_start(out=wq0[:], in_=wqa_d[0:128, :])
            nc.sync.dma_start(out=wq1[:], in_=wqa_d[128:256, :])
            nc.sync.dma_start(out=wk0[:], in_=wka_d[0:128, :])
            nc.sync.dma_start(out=wk1[:], in_=wka_d[128:256, :])
            nc.sync.dma_start(out=wvo0[:], in_=wvoa_d[0:128, :])
            nc.sync.dma_start(out=wvo1[:], in_=wvoa_d[128:256, :])
            if use_bias:
                ones_row = const.tile([1, 512], bf, tag="ones_row")
                nc.gpsimd.memset(ones_row[:], 1.0)
                wqb = const.tile([1, 128], bf, tag="wqb")
                wkb = const.tile([1, 128], bf, tag="wkb")
                wvob = const.tile([1, 256], bf, tag="wvob")
                nc.sync.dma_start(out=wqb[:], in_=wqa_d[256:257, :])
                nc.sync.dma_start(out=wkb[:], in_=wka_d[256:257, :])
                nc.sync.dma_start(out=wvob[:], in_=wvoa_d[256:257, :])

            # Persistent SBUF: replicated qT/kT (4 bands of 32 partitions each
            # hold identical data, enabling PE row tiling), and W (= V@Wo) rows
            # with a ones column at 256 for the softmax denominator.
            qT4 = big.tile([128, NQ], bf, tag="qT4")
            kT4 = big.tile([128, N], bf, tag="kT4")
            wsb = big.tile([128, 16, 2, 260], bf, tag="wsb")
            nc.vector.memset(wsb[:, :, :, 256:257], 1.0)

            # ---- phase B: per 512-column chunk of xT: DMA, q/k proj, W proj ----
            for s in range(8):
                xb = xbp.tile([128, 2, 512], bf, tag="xb")
                nc.sync.dma_start(out=xb[:], in_=xT_d[:, :, 512 * s : 512 * s + 512])
                if s < 4:
                    pq = ps2.tile([128, 512], f32, tag="x", name=f"pq{s}")
                    nc.tensor.matmul(pq[:], wq0[:], xb[:, 0, :], start=True, stop=False)
                    nc.tensor.matmul(pq[:], wq1[:], xb[:, 1, :], start=False, stop=not use_bias)
                    if use_bias:
                        nc.tensor.matmul(pq[:], wqb[:], ones_row[:], start=False, stop=True)
                    nc.vector.tensor_copy(qT4[:, 512 * s : 512 * s + 512], pq[:])
                pk = ps2.tile([128, 512], f32, tag="x", name=f"pk{s}")
                nc.tensor.matmul(pk[:], wk0[:], xb[:, 0, :], start=True, stop=False)
                nc.tensor.matmul(pk[:], wk1[:], xb[:, 1, :], start=False, stop=not use_bias)
                if use_bias:
                    nc.tensor.matmul(pk[:], wkb[:], ones_row[:], start=False, stop=True)
                nc.vector.tensor_copy(kT4[:, 512 * s : 512 * s + 512], pk[:])
                # W = x @ (wv@wo): 4 key chunks of 128 per xb, 2 chunks per psum
                for half in range(2):
                    pw = ps2.tile([128, 512], f32, tag="x", name=f"pw{s}_{half}")
                    for j in range(2):
                        off = 128 * (2 * half + j)
                        nc.tensor.matmul(pw[:, 256 * j : 256 * j + 256], xb[:, 0, off : off + 128], wvo0[:], start=True, stop=False)
                        nc.tensor.matmul(pw[:, 256 * j : 256 * j + 256], xb[:, 1, off : off + 128], wvo1[:], start=False, stop=not use_bias)
                        if use_bias:
                            nc.tensor.matmul(pw[:, 256 * j : 256 * j + 256], ones_row[:, 0:128], wvob[:], start=False, stop=True)
                    P = 2 * s + half
                    # evacuate on scalar engine (vector handles q/k + epilogue)
                    nc.scalar.copy(wsb[:, P, :, 0:256], pw[:])

            # ---- phase C/D: scores (2-way row-tiled) -> exp -> attend ----
            # Superstep u of group g: 4 key chunks m = 4u + 2i + cc
            # (i = substep, cc = concurrent row-tile). Each superstep gets its
            # own [128, 1024] score psum (2 banks; col-half = cc so the two
            # concurrent row-tiles land in different banks); bufs=3 gives a
            # 3-deep ring in 6 banks. Supersteps are processed in PAIRS so the
            # PE switches tile-mode only twice per 8 chunks (16-matmul attend
            # bursts amortize the mode-switch drain).

            def epilogue(qb, pa_t):
                rec = small.tile([128, 1], f32, tag="rec")
                nc.vector.reciprocal(rec[:], pa_t[:, 256:257])
                xq = small.tile([128, 256], f32, tag="xq", bufs=3)
                nc.sync.dma_start(out=xq[:], in_=xq32_d[128 * qb : 128 * qb + 128, :])
                sc = small.tile([128, 256], f32, tag="sc2")
                nc.vector.tensor_scalar(sc[:], pa_t[:, 0:256], rec[:], None, Mult)
                ot = small.tile([128, 256], f32, tag="ot", bufs=3)
                nc.vector.tensor_tensor(ot[:], sc[:], xq[:], Add)
                nc.sync.dma_start(out=out_d[128 * qb : 128 * qb + 128, :], in_=ot[:])

            pa_tiles = {}
            prev_pair = []  # [(et, g, u), ...]
            NP = 8 * 4  # 8 groups x 4 superstep-pairs
            for pidx in range(NP + 1):
                curr_pair = []
                if pidx < NP:
                    g, up = divmod(pidx, 4)
                    for half in range(2):
                        u = 2 * up + half
                        if u == 0:
                            pa_tiles[2 * g] = ps2.tile([128, 512], f32, tag="x", name=f"pa{2 * g}")
                            pa_tiles[2 * g + 1] = ps2.tile([128, 512], f32, tag="x", name=f"pa{2 * g + 1}")
                        pst = ps_sc.tile([128, 1024], f32, tag="sc")
                        for i in range(2):
                            for cc in range(2):
                                m = 4 * u + 2 * i + cc
                                nc.tensor.matmul(
                                    pst[:, 512 * cc + 256 * i : 512 * cc + 256 * i + 256],
                                    kT4[32 * cc : 32 * cc + 32, 128 * m : 128 * m + 128],
                                    qT4[32 * cc : 32 * cc + 32, 256 * g : 256 * g + 256],
                                    start=True,
                                    stop=True,
                                    tile_position=(32 * cc, 0),
                                )
                        et = expp.tile([128, 1024], bf, tag="e")
                        nc.scalar.activation(et[:], pst[:], Exp)
                        curr_pair.append((et, g, u))
                # attend with previous pair's exp tiles (keeps PE busy during exp)
                for et_p, g_p, u_p in prev_pair:
                    for i in range(2):
                        for cc in range(2):
                            m = 4 * u_p + 2 * i + cc
                            for h in range(2):
                                nc.tensor.matmul(
                                    pa_tiles[2 * g_p + h][:, 0:257],
                                    et_p[:, 512 * cc + 256 * i + 128 * h : 512 * cc + 256 * i + 128 * h + 128],
                                    wsb[:, m // 2, cc, 0:257],
                                    start=(m == 0),
                                    stop=(m == KC - 1),
                                )
                    if u_p == 7:
                        for h in range(2):
                            epilogue(2 * g_p + h, pa_tiles[2 * g_p + h])
                            del pa_tiles[2 * g_p + h]
                prev_pair = curr_pair

    nc.compile()
    return nc


def _get_compiled(use_bias: bool):
    key = bool(use_bias)
    if key not in _compiled_cache:
        _compiled_cache[key] = _build(use_bias)
    return _compiled_cache[key]


def _prep(x, wq, bq, wk, bk, wv, bv, wo, bo):
    xf = np.ascontiguousarray(np.asarray(x, dtype=np.float32)).reshape(B, N, C)
    wq = np.asarray(wq, np.float32)
    bq = np.asarray(bq, np.float32)
    wk = np.asarray(wk, np.float32)
    bk = np.asarray(bk, np.float32)
    wv = np.asarray(wv, np.float32)
    bv = np.asarray(bv, np.float32)
    wo = np.asarray(wo, np.float32)
    bo = np.asarray(bo, np.float32)

    use_bias = not (
        np.all(bq == 0) and np.all(bk == 0) and np.all(bv == 0) and np.all(bo == 0)
    )

    scale = np.float32(1.0 / np.sqrt(np.float32(D)))
    wqa = np.concatenate([wq, bq[None, :]], 0) * scale  # fold softmax scale into q
    wka = np.concatenate([wk, bk[None, :]], 0)
    wqa_rep = np.ascontiguousarray(np.tile(wqa, (1, 4))).astype(BF16)  # [257, 128]
    wka_rep = np.ascontiguousarray(np.tile(wka, (1, 4))).astype(BF16)
    # fold wo into the value projection: W = x @ (wv@wo) + bv@wo
    wvo = wv @ wo
    bvo = bv @ wo
    wvoa = np.ascontiguousarray(
        np.concatenate([wvo, bvo[None, :]], 0)
    ).astype(BF16)  # [257, 256]

    in_maps = []
    for core in range(NCORES):
        b, h = divmod(core, 2)
        if h == 0:
            xo = xf[b]
        else:
            xo = np.concatenate([xf[b, NQ:], xf[b, :NQ]], 0)
        # channel-major transpose on host: [256, 4096] -> [128, 2, 4096]
        xT = np.ascontiguousarray(
            xo.T.reshape(2, 128, N).transpose(1, 0, 2).astype(BF16)
        )
        xq = np.ascontiguousarray(xo[:NQ])
        if use_bias:
            xq = xq + bo[None, :]
        in_maps.append(
            {
                "xT": xT,
                "xq32": xq,
                "wqa_rep": wqa_rep,
                "wka_rep": wka_rep,
                "wvoa": wvoa,
            }
        )
    return in_maps, use_bias


def _gather(results):
    out = np.empty((B, N, C), np.float32)
    for core in range(NCORES):
        b, h = divmod(core, 2)
        out[b, NQ * h : NQ * (h + 1)] = results[core]["out"]
    return out.reshape(B, HH, WW, C)


def kernel(x, wq, bq, wk, bk, wv, bv, wo, bo):
    from concourse.bass_utils import run_bass_kernel_spmd

    in_maps, use_bias = _prep(x, wq, bq, wk, bk, wv, bv, wo, bo)
    nc = _get_compiled(use_bias)
    res = run_bass_kernel_spmd(nc, in_maps, core_ids=list(range(NCORES)))
    return _gather(res.results)


def _ensure_ntff_hook():
    """The agent image's antenv stub lacks axon_hooks; synthesize it so
    run_bass_kernel_spmd(trace=True) can NTFF-profile via libaxon_pjrt."""
    import types

    try:
        from antenv.axon_hooks import get_axon_ntff_profile_hook  # noqa: F401
        return
    except ImportError:
        pass
    import antenv
    from trn_agent_boot.trn_boot import _ntff_profile_via_ctypes

    mod = types.ModuleType("antenv.axon_hooks")
    state = {"h": _ntff_profile_via_ctypes("/opt/axon/libaxon_pjrt.so")}
    mod.get_axon_ntff_profile_hook = lambda: state["h"]
    mod.set_axon_ntff_profile_hook = lambda h: state.__setitem__("h", h)
    sys.modules["antenv.axon_hooks"] = mod
    antenv.axon_hooks = mod


def run_traced(inputs, **kw):
    """For test.py: run with NTFF profiling; returns (output, BassKernelResults)."""
    from concourse.bass_utils import run_bass_kernel_spmd

    _ensure_ntff_hook()

    in_maps, use_bias = _prep(**inputs)
    nc = _get_compiled(use_bias)
    res = run_bass_kernel_spmd(nc, in_maps, core_ids=list(range(NCORES)), trace=True, **kw)
    return _gather(res.results), res
